# revision 1
# baseline (speedup 1.0000x reference)
"""KAN layer (piecewise-linear spline edges) as a Trainium2 Bass kernel.

Math: y[b,o] = sum_i lerp(S[o,i,:], u) + bias[o],  u = (clip(x[b,i]*W[o,i],-1,1)+1)*7.5

Key transformation: for each edge (o,i), f_{o,i}(x) = that lerp as a function of
x is piecewise-linear in x. We resample every edge function onto one SHARED
uniform x-grid of GX points (exact on affine pieces; kink resample error
~1e-3 rel for GX=128 given |W|<=1/16 here). Then

    y[b,o] = sum_{i,h} PHI[o,i,h] * hat_h(x[b,i])

which is a dense matmul over K=(i,h) — no per-element gathers. hat_h(x[b,i])
is built on-chip: PE replicates x across partitions (via 0/1-pattern matmuls,
bf16 hi+lo split for fp32 accuracy), ACT evaluates the hat with per-partition
bias in 2 activation ops. The table PHI depends only on weights, so it is
precomputed host-side (weight repacking), batch-data work all runs on HW.

Sharding: data-parallel over batch, 8 cores x 128 rows; PHI replicated.
"""

import numpy as np
import ml_dtypes

import concourse.bacc as bacc
import concourse.bass as bass
import concourse.mybir as mybir
import concourse.tile as tile
from concourse.bass_utils import run_bass_kernel_spmd

B, IN, OUT, G = 1024, 256, 256, 16
GX = 128               # shared x-grid size
NC_N = 8               # cores
BS = B // NC_N         # batch rows per core
KT = (IN * GX) // 128  # 256 K-tiles
AF = np.dtype(ml_dtypes.bfloat16)

_PROG_CACHE = {}


def _build_program():
    nc = bacc.Bacc(
        "TRN2",
        target_bir_lowering=False,
        debug=False,
        enable_asserts=False,
        num_devices=NC_N,
    )
    f32 = mybir.dt.float32
    bf16 = mybir.dt.bfloat16

    xthi_d = nc.dram_tensor("xthi", [2, 128, BS], bf16, kind="ExternalInput")
    xtlo_d = nc.dram_tensor("xtlo", [2, 128, BS], bf16, kind="ExternalInput")
    pats_d = nc.dram_tensor("pats", [16, 128, 128], bf16, kind="ExternalInput")
    hb_d = nc.dram_tensor("hb", [128, 8], f32, kind="ExternalInput")
    scl_d = nc.dram_tensor("scl", [128, 1], f32, kind="ExternalInput")
    atab_d = nc.dram_tensor("atab", [KT, 128, OUT], bf16, kind="ExternalInput")
    y_d = nc.dram_tensor("y", [BS, OUT], f32, kind="ExternalOutput")

    Act = mybir.ActivationFunctionType

    with tile.TileContext(nc) as tc:
        with (
            tc.tile_pool(name="const", bufs=1) as cp,
            tc.tile_pool(name="psx", bufs=2, space="PSUM") as psx,
            tc.tile_pool(name="psy", bufs=1, space="PSUM") as psy,
            tc.tile_pool(name="tmp", bufs=3) as tp,
            tc.tile_pool(name="hp", bufs=3) as hp,
            tc.tile_pool(name="ap", bufs=6) as apl,
        ):
            xthi = cp.tile([128, 2 * BS], bf16)
            xtlo = cp.tile([128, 2 * BS], bf16)
            for t in range(2):
                nc.sync.dma_start(xthi[:, t * BS:(t + 1) * BS], xthi_d.ap()[t])
                nc.sync.dma_start(xtlo[:, t * BS:(t + 1) * BS], xtlo_d.ap()[t])
            pats = cp.tile([128, 16 * 128], bf16)
            for q in range(16):
                nc.sync.dma_start(pats[:, q * 128:(q + 1) * 128], pats_d.ap()[q])
            hb = cp.tile([128, 8], f32)
            nc.sync.dma_start(hb, hb_d.ap())
            scl = cp.tile([128, 1], f32)
            nc.sync.dma_start(scl, scl_d.ap())

            py = psy.tile([128, OUT], f32)
            kt = 0
            for hh in range(8):
                for grp in range(8):
                    px = psx.tile([128, 4 * BS], f32)
                    for j in range(4):
                        ih = grp * 4 + j
                        q, src = ih % 16, ih // 16
                        sl = px[:, j * BS:(j + 1) * BS]
                        nc.tensor.matmul(
                            sl,
                            lhsT=pats[:, q * 128:(q + 1) * 128],
                            rhs=xthi[:, src * BS:(src + 1) * BS],
                            start=True, stop=False, skip_group_check=True,
                        )
                        nc.tensor.matmul(
                            sl,
                            lhsT=pats[:, q * 128:(q + 1) * 128],
                            rhs=xtlo[:, src * BS:(src + 1) * BS],
                            start=False, stop=True, skip_group_check=True,
                        )
                    tmp = tp.tile([128, 4 * BS], f32)
                    nc.scalar.activation(tmp, px, Act.Abs,
                                         bias=hb[:, hh:hh + 1], scale=scl[:, 0:1])
                    ht = hp.tile([128, 4 * BS], bf16)
                    nc.scalar.activation(ht, tmp, Act.Relu, bias=1.0, scale=-1.0)
                    for j in range(4):
                        at = apl.tile([128, OUT], bf16, tag="A")
                        nc.sync.dma_start(at, atab_d.ap()[kt])
                        nc.tensor.matmul(
                            py,
                            lhsT=ht[:, j * BS:(j + 1) * BS],
                            rhs=at,
                            start=(kt == 0), stop=(kt == KT - 1),
                            skip_group_check=True,
                        )
                        kt += 1
            yt = tp.tile([128, OUT], f32, tag="y")
            nc.vector.tensor_copy(yt, py)
            nc.sync.dma_start(y_d.ap(), yt)

    nc.compile()
    return nc


def _edge_table(W, S, bias, xs):
    """PHI[o,i,h] = edge function evaluated at grid xs (float64), bias folded."""
    Wf = W.reshape(-1, 1).astype(np.float64)
    Sf = S.reshape(-1, G).astype(np.float64)
    tt = np.clip(Wf * xs[None, :], -1.0, 1.0)
    uu = (tt + 1.0) * (0.5 * (G - 1))
    idx = np.clip(np.floor(uu).astype(np.int64), 0, G - 2)
    frac = uu - idx
    ar = np.arange(Sf.shape[0])[:, None]
    phi = Sf[ar, idx] + frac * (Sf[ar, idx + 1] - Sf[ar, idx])
    phi = phi.reshape(OUT, IN, GX)
    phi += bias.astype(np.float64)[:, None, None] / IN
    return phi


def kernel(x, W, spline_values, bias, _trace=False):
    x = np.ascontiguousarray(np.asarray(x, dtype=np.float32))
    W = np.asarray(W, dtype=np.float32)
    S = np.asarray(spline_values, dtype=np.float32)
    bias = np.asarray(bias, dtype=np.float32)

    xmax = float(np.abs(x).max()) * (1.0 + 1e-6) + 1e-30
    dx = 2.0 * xmax / (GX - 1)
    xs = np.linspace(-xmax, xmax, GX)

    phi = _edge_table(W, S, bias, xs)
    # K-order: tile t = h_hi*32 + i_hi ; partition p = i_lo*16 + h_lo
    t6 = phi.reshape(OUT, 32, 8, 8, 16).transpose(3, 1, 2, 4, 0)
    atab = np.ascontiguousarray(t6.reshape(KT, 128, OUT)).astype(AF)

    pats = np.zeros((16, 128, 128), np.float32)
    for q in range(16):
        for m in range(128):
            pats[q, q * 8 + m // 16, m] = 1.0
    pats = pats.astype(AF)

    p_idx = np.arange(128)
    hb = (63.5 - (np.arange(8)[None, :] * 16 + (p_idx % 16)[:, None])).astype(np.float32)
    scl = np.full((128, 1), 1.0 / dx, np.float32)

    in_maps = []
    for c in range(NC_N):
        xT = x[c * BS:(c + 1) * BS, :].T  # [IN, BS] f32
        xhi = xT.astype(AF)
        xlo = (xT - xhi.astype(np.float32)).astype(AF)
        in_maps.append({
            "xthi": np.ascontiguousarray(xhi.reshape(2, 128, BS)),
            "xtlo": np.ascontiguousarray(xlo.reshape(2, 128, BS)),
            "pats": pats,
            "hb": hb,
            "scl": scl,
            "atab": atab,
        })

    key = "prog"
    if key not in _PROG_CACHE:
        _PROG_CACHE[key] = _build_program()
    nc = _PROG_CACHE[key]

    res = run_bass_kernel_spmd(
        nc, in_maps, core_ids=list(range(NC_N)), trace=bool(_trace)
    )
    y = np.concatenate([res.results[c]["y"] for c in range(NC_N)], axis=0)
    if _trace:
        kernel._last_result = res
    return y.astype(np.float32)


if __name__ == "__main__":
    rng = np.random.default_rng(0)
    x = rng.standard_normal((B, IN)).astype(np.float32)
    W = (rng.uniform(-1, 1, (OUT, IN)) / np.sqrt(IN)).astype(np.float32)
    S = rng.standard_normal((OUT, IN, G)).astype(np.float32)
    b = np.zeros(OUT, np.float32)
    y = kernel(x, W, S, b)
    print("y", y.shape, y.dtype)



# revision 8
# speedup vs baseline: 6.6050x; 6.6050x over previous
"""KAN layer (piecewise-linear spline edges) as a Trainium2 Bass kernel.

Math: y[b,o] = sum_i lerp(S[o,i,:], u) + bias[o],  u = (clip(x[b,i]*W[o,i],-1,1)+1)*7.5

Key transformation: for each edge (o,i), f_{o,i}(x) is piecewise-linear in x.
We L2-project every edge function onto one SHARED uniform x-grid of GX=32
points (projection roughly halves the kink resample error vs interpolation;
measured ~6e-3 rel end-to-end). With the telescoping identity

    lerp(phi, u) = phi[0] + sum_h (phi[h+1]-phi[h]) * clamp01(u - h)

the batch work becomes  y[b,o] = sum_{i,h} C[o,i,h] * clamp01(u[b,i] - h),
a dense [B,K]x[K,OUT] matmul with K = IN*(GX-1) — no gathers. The clamp01
basis needs one Relu (ACT, per-partition bias) + one min (DVE) instead of
the 2-ACT hat construction. x is replicated across partitions by 32-row
0/1-pattern matmuls (cheap LDWEIGHTS). The constant term phi[0]-sum rides a
padding partition whose basis is forced to 1. The coefficient table C
depends only on weights, so it is precomputed host-side (weight repacking);
batch-data work all runs on HW.

Sharding: data-parallel over batch, 8 cores x 128 rows; C replicated.
"""

import numpy as np
import ml_dtypes

import concourse.bacc as bacc
import concourse.bass as bass
import concourse.mybir as mybir
import concourse.tile as tile
from concourse.bass_utils import run_bass_kernel_spmd

B, IN, OUT, G = 1024, 256, 256, 16
GX = 32                # shared x-grid size
NB = GX - 1            # basis ramps per feature
FPT = 4                # features per 128-partition K-tile
KT = IN // FPT         # 64 K-tiles
NG = KT // 4           # 16 pipeline groups of 4 K-tiles
NC_N = 8               # cores
BS = B // NC_N         # 128 batch rows per core
AF = np.dtype(ml_dtypes.bfloat16)

_PROG_CACHE = {}


def _build_program():
    nc = bacc.Bacc(
        "TRN2",
        target_bir_lowering=False,
        debug=False,
        enable_asserts=False,
        num_devices=NC_N,
    )
    f32 = mybir.dt.float32
    bf16 = mybir.dt.bfloat16

    xb_d = nc.dram_tensor("xb", [128, 2 * BS], bf16, kind="ExternalInput")
    pats_d = nc.dram_tensor("pats", [128, 16 * 128], bf16, kind="ExternalInput")
    sb_d = nc.dram_tensor("sb", [128, 2], f32, kind="ExternalInput")
    atab_d = nc.dram_tensor("atab", [128, KT * OUT], bf16, kind="ExternalInput")
    y_d = nc.dram_tensor("y", [BS, OUT], f32, kind="ExternalOutput")

    Act = mybir.ActivationFunctionType

    with tile.TileContext(nc) as tc:
        with (
            tc.tile_pool(name="const", bufs=1) as cp,
            tc.tile_pool(name="psx", bufs=3, space="PSUM") as psx,
            tc.tile_pool(name="psy", bufs=1, space="PSUM") as psy,
            tc.tile_pool(name="hp", bufs=3) as hp,
        ):
            pats = cp.tile([128, 16 * 128], bf16)
            nc.sync.dma_start(pats, pats_d.ap())
            xb = cp.tile([128, 2 * BS], bf16)
            nc.sync.dma_start(xb, xb_d.ap())
            sb = cp.tile([128, 2], f32)
            nc.sync.dma_start(sb, sb_d.ap())
            atab = cp.tile([128, KT * OUT], bf16)
            NCH = 8
            CW = KT * OUT // NCH
            for ch in range(NCH):
                nc.sync.dma_start(
                    atab[:, ch * CW:(ch + 1) * CW],
                    atab_d.ap()[:, ch * CW:(ch + 1) * CW],
                )

            py = psy.tile([128, OUT], f32)

            def accum(g, ht):
                for j in range(4):
                    kt = g * 4 + j
                    nc.tensor.matmul(
                        py,
                        lhsT=ht[:, j * BS:(j + 1) * BS],
                        rhs=atab[:, kt * OUT:(kt + 1) * OUT],
                        start=(kt == 0), stop=(kt == KT - 1),
                        skip_group_check=True,
                    )

            prev = None
            for g in range(NG):
                px = psx.tile([128, 4 * BS], f32)
                for j in range(4):
                    kt = g * 4 + j
                    q, src = kt % 32, kt // 32
                    blk = (q // 16) * 64
                    qq = q % 16
                    nc.tensor.matmul(
                        px[:, j * BS:(j + 1) * BS],
                        lhsT=pats[blk:blk + 64, qq * 128:(qq + 1) * 128],
                        rhs=xb[blk:blk + 64, src * BS:(src + 1) * BS],
                        start=True, stop=True, skip_group_check=True,
                    )
                tmp = hp.tile([128, 4 * BS], bf16, tag="tmp")
                nc.scalar.activation(tmp, px, Act.Relu,
                                     bias=sb[:, 0:1], scale=sb[:, 1:2])
                ht = hp.tile([128, 4 * BS], bf16, tag="ht")
                nc.vector.tensor_scalar_min(ht, tmp, 1.0)
                if prev is not None:
                    accum(*prev)
                prev = (g, ht)
            accum(*prev)

            yt = hp.tile([128, OUT], f32, tag="y")
            nc.vector.tensor_copy(yt, py)
            nc.sync.dma_start(y_d.ap(), yt)

    nc.compile()
    return nc


def _edge_table_fine(W, S, xs):
    """Edge functions evaluated at points xs (float64). [OUT*IN, len(xs)]"""
    Wf = W.reshape(-1, 1).astype(np.float64)
    Sf = S.reshape(-1, G).astype(np.float64)
    tt = np.clip(Wf * xs[None, :], -1.0, 1.0)
    uu = (tt + 1.0) * (0.5 * (G - 1))
    idx = np.clip(np.floor(uu).astype(np.int64), 0, G - 2)
    frac = uu - idx
    ar = np.arange(Sf.shape[0])[:, None]
    return Sf[ar, idx] + frac * (Sf[ar, idx + 1] - Sf[ar, idx])


def _build_tables(x, W, S, bias):
    xmax = float(np.abs(x).max()) * (1.0 + 1e-6) + 1e-30
    dx = 2.0 * xmax / (GX - 1)
    FINE = 8
    GF = (GX - 1) * FINE + 1
    xf = np.linspace(-xmax, xmax, GF)
    F = _edge_table_fine(W, S, xf)                       # [E, GF]
    u = (xf + xmax) / dx
    Hb = np.maximum(0.0, 1.0 - np.abs(u[None, :] - np.arange(GX)[:, None]))
    wq = np.full(GF, 1.0)
    wq[0] = wq[-1] = 0.5
    Hw = Hb * wq[None, :]
    phi = np.linalg.solve(Hw @ Hb.T, (F @ Hw.T).T).T     # [E, GX] L2 projection
    phi = phi.reshape(OUT, IN, GX)
    c = np.diff(phi, axis=2)                             # [OUT, IN, NB]
    offset = phi[:, :, 0].sum(axis=1) + bias.astype(np.float64)
    pack = np.zeros((KT, FPT, GX, OUT), np.float64)
    pack[:, :, :NB, :] = c.transpose(1, 2, 0).reshape(KT, FPT, NB, OUT)
    pack[0, 0, NB, :] = offset
    atab = np.ascontiguousarray(
        pack.reshape(KT, 128, OUT).transpose(1, 0, 2).reshape(128, KT * OUT)
    ).astype(AF)

    p = np.arange(128)
    h = p % GX
    bias_v = np.where(h == NB, 1.0, xmax / dx - h).astype(np.float32)
    scale_v = np.full(128, 1.0 / dx, np.float32)
    sb = np.ascontiguousarray(np.stack([bias_v, scale_v], axis=1))
    return atab, sb


def _build_pats():
    r = np.arange(128)[:, None]
    pcol = np.arange(128)[None, :]
    pats = np.zeros((128, 16 * 128), np.float32)
    for qq in range(16):
        pats[:, qq * 128:(qq + 1) * 128] = (
            (r % 64) == (qq * 4 + pcol // 32)
        ).astype(np.float32)
    # pad slots (h == NB) read no x: their basis is forced to 1 via the
    # ACT bias so one of them can carry the constant/offset table row
    pats[:, (np.arange(16 * 128) % GX) == NB] = 0.0
    return pats.astype(AF)


def kernel(x, W, spline_values, bias, _trace=False):
    x = np.ascontiguousarray(np.asarray(x, dtype=np.float32))
    W = np.asarray(W, dtype=np.float32)
    S = np.asarray(spline_values, dtype=np.float32)
    bias = np.asarray(bias, dtype=np.float32)

    atab, sb = _build_tables(x, W, S, bias)
    pats = _build_pats()

    in_maps = []
    for c in range(NC_N):
        xT = x[c * BS:(c + 1) * BS, :].T                 # [IN, BS]
        xb = np.ascontiguousarray(
            xT.reshape(2, 128, BS).transpose(1, 0, 2).reshape(128, 2 * BS)
        ).astype(AF)
        in_maps.append({"xb": xb, "pats": pats, "sb": sb, "atab": atab})

    key = "prog"
    if key not in _PROG_CACHE:
        _PROG_CACHE[key] = _build_program()
    nc = _PROG_CACHE[key]

    res = run_bass_kernel_spmd(
        nc, in_maps, core_ids=list(range(NC_N)), trace=bool(_trace)
    )
    y = np.concatenate([res.results[c]["y"] for c in range(NC_N)], axis=0)
    if _trace:
        kernel._last_result = res
    return y.astype(np.float32)


if __name__ == "__main__":
    rng = np.random.default_rng(0)
    x = rng.standard_normal((B, IN)).astype(np.float32)
    W = (rng.uniform(-1, 1, (OUT, IN)) / np.sqrt(IN)).astype(np.float32)
    S = rng.standard_normal((OUT, IN, G)).astype(np.float32)
    b = np.zeros(OUT, np.float32)
    y = kernel(x, W, S, b)
    print("y", y.shape, y.dtype)
